# revision 3
# baseline (speedup 1.0000x reference)
"""LoLa message-passing kernel for 8 Trainium2 NeuronCores.

Math (algebraically identical to the reference, verified to ~3e-7 rel):
  out[..., 0] = masses      = f3^2 - f0^2 - f1^2 - f2^2
  out[..., 1] = ptsq        = f1^2 + f2^2
  out[..., 2] = w_ener   @ f0
  out[..., 3] = weighted_d  = masses * rowsum(w_dist) + w_dist @ masses
                              + 2*(f0*(w_dist@f0) + f1*(w_dist@f1)
                                   + f2*(w_dist@f2) - f3*(w_dist@f3))
  out[..., 4] = w_pid    @ f3
  out[..., 5] = w_extra0 @ f4
  out[..., 6] = w_extra1 @ f5

Sharding: model-parallel over the particle axis N (64 output rows per core);
combvec is replicated (needed in full as the contraction operand), weights are
sliced 1/8 per core.  All matmul operands are laid out with the contraction
index m on SBUF partitions, which requires W^T and combvec^T — both produced
on the host (numpy) so the device kernel does zero transposes.
"""

import sys

if "/opt/trn_rl_repo" not in sys.path:
    sys.path.insert(0, "/opt/trn_rl_repo")

import numpy as np

import concourse.bass as bass
import concourse.mybir as mybir
import concourse.tile as tile
from concourse import bacc
from concourse.bass_utils import run_bass_kernel_spmd

B, N, F = 128, 512, 6
NCORES = 8
NS = N // NCORES  # 64 output rows per core
KC = N // 128  # 4 contraction chunks of 128
DT = mybir.dt.float32
ALU = mybir.AluOpType
ACTF = mybir.ActivationFunctionType

# feature index used as rhs for each of the 4 plain mixes, in weight order
# wt layout: [0]=w_dist, [1]=w_ener, [2]=w_pid, [3]=w_extra0, [4]=w_extra1
MIX_FEAT = {1: 0, 2: 3, 3: 4, 4: 5}
# output channel for each mix weight: w_ener->2, w_pid->4, w_extra0->5, w_extra1->6
MIX_CHAN = {1: 2, 2: 4, 3: 5, 4: 6}


def _emit(tc, nc, ft_d, wt_d, fr_d, out_d):
    with (
        tc.tile_pool(name="sbuf", bufs=1) as sb,
        tc.tile_pool(name="psum", bufs=1, space="PSUM") as ps,
    ):
        # --- persistent SBUF tiles ---
        ft = sb.tile([128, F * N], DT)  # feature k, chunk c at [k*512 + c*128]
        wt = sb.tile([128, 5 * KC * NS], DT)  # weight i, chunk c at [(i*4+c)*64]
        fr = sb.tile([64, 4 * B], DT)  # this core's n-rows of f0..f3
        sq = sb.tile([128, 4 * N], DT)  # squares of f0..f3
        mass = sb.tile([128, KC * 129], DT)  # chunk c at [c*129], col c*129+128 = 1.0
        frsq = sb.tile([64, 4 * B], DT)
        quad = sb.tile([64, 4 * B], DT)
        qs = sb.tile([64, 2 * B], DT)
        outs = sb.tile([64, 7 * B], DT)  # output channels staging

        # --- PSUM tiles ---
        psA = ps.tile([64, 4 * B], DT)  # w_dist @ [f0|f1|f2|f3]
        psB = ps.tile([64, B + 1], DT)  # [w_dist @ masses | rowsum]
        psY = [ps.tile([64, B], DT, name=f"psY{i}") for i in range(1, 5)]

        # --- DMAs in (emitted chunk-major so early chunks land first) ---
        for k in range(4):
            nc.sync.dma_start(fr[:, k * B:(k + 1) * B], fr_d[k, :, :])
        for c in range(KC):
            for k in range(F):
                nc.sync.dma_start(
                    ft[:, k * N + c * 128: k * N + (c + 1) * 128],
                    ft_d[k, c * 128:(c + 1) * 128, :],
                )
            for i in range(5):
                nc.sync.dma_start(
                    wt[:, (i * KC + c) * NS: (i * KC + c + 1) * NS],
                    wt_d[i, c * 128:(c + 1) * 128, :],
                )

        # --- masses (full, transposed layout) = f3^2 - f2^2 - f1^2 - f0^2 ---
        # squares of features 0..3 (ACT does half, DVE the other half)
        nc.scalar.activation(sq[:, 0: 2 * N], ft[:, 0: 2 * N], ACTF.Square)
        nc.vector.tensor_tensor(
            out=sq[:, 2 * N: 4 * N],
            in0=ft[:, 2 * N: 4 * N],
            in1=ft[:, 2 * N: 4 * N],
            op=ALU.mult,
        )
        m3 = mass[:].rearrange("p (c x) -> p c x", c=KC, x=129)
        mview = m3[:, :, 0:128]
        sq4 = sq[:].rearrange("p (k c b) -> p k c b", k=4, c=KC, b=B)
        nc.vector.tensor_tensor(out=mview, in0=sq4[:, 3], in1=sq4[:, 2], op=ALU.subtract)
        nc.vector.tensor_tensor(out=mview, in0=mview, in1=sq4[:, 1], op=ALU.subtract)
        nc.vector.tensor_tensor(out=mview, in0=mview, in1=sq4[:, 0], op=ALU.subtract)
        nc.vector.memset(m3[:, :, 128:129], 1.0)

        # --- this core's row-slice: masses_R (ch0) and ptsq (ch1) ---
        nc.vector.tensor_tensor(out=frsq[:], in0=fr[:], in1=fr[:], op=ALU.mult)
        nc.vector.tensor_tensor(
            out=outs[:, 0:B], in0=frsq[:, 3 * B: 4 * B], in1=frsq[:, 2 * B: 3 * B],
            op=ALU.subtract,
        )
        nc.vector.tensor_tensor(
            out=outs[:, 0:B], in0=outs[:, 0:B], in1=frsq[:, B: 2 * B], op=ALU.subtract
        )
        nc.vector.tensor_tensor(
            out=outs[:, 0:B], in0=outs[:, 0:B], in1=frsq[:, 0:B], op=ALU.subtract
        )
        nc.vector.tensor_tensor(
            out=outs[:, B: 2 * B], in0=frsq[:, B: 2 * B], in1=frsq[:, 2 * B: 3 * B],
            op=ALU.add,
        )

        # --- matmuls: contraction over m in 4 chunks of 128 ---
        def wslice(i, c):
            return wt[:, (i * KC + c) * NS: (i * KC + c + 1) * NS]

        def fslice(k, c):
            return ft[:, k * N + c * 128: k * N + (c + 1) * 128]

        ft4 = ft[:].rearrange("p (k c b) -> p k c b", k=F, c=KC, b=B)
        for c in range(KC):
            st, sp = c == 0, c == KC - 1
            # w_dist @ [f0|f1|f2|f3] -> psA, one 512-wide moving operand
            nc.tensor.matmul(
                psA[:], wslice(0, c), ft4[:, 0:4, c, :], start=st, stop=sp
            )
            # plain mixes
            for i in range(1, 5):
                nc.tensor.matmul(
                    psY[i - 1][:], wslice(i, c), fslice(MIX_FEAT[i], c),
                    start=st, stop=sp,
                )
        # w_dist @ [masses | ones] (needs masses, so emitted after; own group)
        for c in range(KC):
            nc.tensor.matmul(
                psB[:], wslice(0, c), mass[:, c * 129:(c + 1) * 129],
                start=c == 0, stop=c == KC - 1,
            )

        # --- epilogue ---
        # quad_k = fr_k * (w_dist @ f_k), k-major layouts line up
        nc.vector.tensor_tensor(out=quad[:], in0=fr[:], in1=psA[:], op=ALU.mult)
        nc.vector.tensor_tensor(
            out=qs[:, 0:B], in0=quad[:, 0:B], in1=quad[:, B: 2 * B], op=ALU.add
        )
        nc.vector.tensor_tensor(
            out=qs[:, B: 2 * B], in0=quad[:, 2 * B: 3 * B], in1=quad[:, 3 * B: 4 * B],
            op=ALU.subtract,
        )
        nc.vector.tensor_tensor(
            out=qs[:, 0:B], in0=qs[:, 0:B], in1=qs[:, B: 2 * B], op=ALU.add
        )
        # wd = masses_R * rowsum + (w_dist @ masses)
        nc.vector.scalar_tensor_tensor(
            out=qs[:, B: 2 * B],
            in0=outs[:, 0:B],
            scalar=psB[:, B: B + 1],
            in1=psB[:, 0:B],
            op0=ALU.mult,
            op1=ALU.add,
        )
        # ch3 = wd + 2*quad
        nc.vector.scalar_tensor_tensor(
            out=outs[:, 3 * B: 4 * B],
            in0=qs[:, 0:B],
            scalar=2.0,
            in1=qs[:, B: 2 * B],
            op0=ALU.mult,
            op1=ALU.add,
        )
        # plain mix channels: PSUM -> SBUF staging
        for i in range(1, 5):
            ch = MIX_CHAN[i]
            nc.scalar.copy(outs[:, ch * B:(ch + 1) * B], psY[i - 1][:])

        # --- DMAs out, one per channel ---
        for ch in range(7):
            nc.sync.dma_start(out_d[ch, :, :], outs[:, ch * B:(ch + 1) * B])


_NC_CACHE = {}


def _get_nc():
    if "nc" not in _NC_CACHE:
        nc = bacc.Bacc(
            "TRN2", target_bir_lowering=False, debug=False, num_devices=NCORES
        )
        ft_d = nc.dram_tensor("ft", [F, N, B], DT, kind="ExternalInput")
        wt_d = nc.dram_tensor("wt", [5, N, NS], DT, kind="ExternalInput")
        fr_d = nc.dram_tensor("fr", [4, NS, B], DT, kind="ExternalInput")
        out_d = nc.dram_tensor("out", [7, NS, B], DT, kind="ExternalOutput")
        with tile.TileContext(nc) as tc:
            _emit(tc, nc, ft_d.ap(), wt_d.ap(), fr_d.ap(), out_d.ap())
        nc.compile()
        _NC_CACHE["nc"] = nc
    return _NC_CACHE["nc"]


def make_in_maps(combvec, w_dist, w_ener, w_pid, w_extra0, w_extra1):
    ftr = np.ascontiguousarray(np.transpose(combvec, (2, 1, 0)), dtype=np.float32)
    weights = (w_dist, w_ener, w_pid, w_extra0, w_extra1)
    in_maps = []
    for c in range(NCORES):
        sl = slice(NS * c, NS * (c + 1))
        wtc = np.stack(
            [np.ascontiguousarray(np.asarray(w, np.float32)[sl].T) for w in weights]
        )
        frc = np.ascontiguousarray(ftr[:4, sl, :])
        in_maps.append({"ft": ftr, "wt": wtc, "fr": frc})
    return in_maps


def kernel(combvec, w_dist, w_ener, w_pid, w_extra0, w_extra1, _bench=None):
    in_maps = make_in_maps(combvec, w_dist, w_ener, w_pid, w_extra0, w_extra1)
    nc = _get_nc()
    kw = dict(_bench) if _bench else {}
    res = run_bass_kernel_spmd(nc, in_maps, core_ids=list(range(NCORES)), **kw)
    full = np.concatenate([r["out"] for r in res.results], axis=1)  # (7, 512, 128)
    out = np.ascontiguousarray(np.transpose(full, (2, 1, 0)))  # (128, 512, 7)
    if _bench is not None:
        kernel.last_results = res
    return out


# revision 8
# speedup vs baseline: 1.1404x; 1.1404x over previous
"""LoLa message-passing kernel for 8 Trainium2 NeuronCores.

Math (algebraically identical to the reference, verified to ~3e-7 rel):
  out[..., 0] = masses      = f3^2 - f0^2 - f1^2 - f2^2
  out[..., 1] = ptsq        = f1^2 + f2^2
  out[..., 2] = w_ener   @ f0
  out[..., 3] = weighted_d  = masses * rowsum(w_dist) + w_dist @ masses
                              + 2*(f0*(w_dist@f0) + f1*(w_dist@f1)
                                   + f2*(w_dist@f2) - f3*(w_dist@f3))
  out[..., 4] = w_pid    @ f3
  out[..., 5] = w_extra0 @ f4
  out[..., 6] = w_extra1 @ f5

Sharding: model-parallel over the particle axis N (64 output rows per core);
combvec is replicated (needed in full as the contraction operand), weights are
sliced 1/8 per core.  All matmul operands are laid out with the contraction
index m on SBUF partitions, which requires W^T and combvec^T — both produced
on the host (numpy) so the device kernel does zero transposes.
"""

import sys

if "/opt/trn_rl_repo" not in sys.path:
    sys.path.insert(0, "/opt/trn_rl_repo")

import numpy as np

import concourse.bass as bass
import concourse.mybir as mybir
import concourse.tile as tile
from concourse import bacc
from concourse.bass_utils import run_bass_kernel_spmd

B, N, F = 128, 512, 6
NCORES = 8
NS = N // NCORES  # 64 output rows per core
KC = N // 128  # 4 contraction chunks of 128
DT = mybir.dt.float32
DTR = mybir.dt.float32r
ALU = mybir.AluOpType
ACTF = mybir.ActivationFunctionType

# feature index used as rhs for each of the 4 plain mixes, in weight order
# wt layout: [0]=w_dist, [1]=w_ener, [2]=w_pid, [3]=w_extra0, [4]=w_extra1
MIX_FEAT = {1: 0, 2: 3, 3: 4, 4: 5}
# output channel for each mix weight: w_ener->2, w_pid->4, w_extra0->5, w_extra1->6
MIX_CHAN = {1: 2, 2: 4, 3: 5, 4: 6}


def _emit(tc, nc, ft_d, wt_d, fr_d, out_d):
    with (
        tc.tile_pool(name="sbuf", bufs=1) as sb,
        tc.tile_pool(name="psum", bufs=1, space="PSUM") as ps,
    ):
        # --- persistent SBUF tiles ---
        ft = sb.tile([128, F * N], DTR)  # feature k, chunk c at [k*512 + c*128]
        wt = sb.tile([128, 5 * KC * NS], DTR)  # weight i, chunk c at [(i*4+c)*64]
        fr = sb.tile([64, 4 * B], DT)  # this core's n-rows of f0..f3
        sq = sb.tile([128, 4 * N], DT)  # squares of f0..f3
        mass = sb.tile([128, KC * 130], DTR)  # chunk c at [c*130]; col 128=1.0, 129=0 pad
        onez = sb.tile([128, 2], DT)  # [1.0, 0.0] broadcast source
        frsq = sb.tile([64, 4 * B], DT)
        quad = sb.tile([64, 4 * B], DT)
        qs = sb.tile([64, 2 * B], DT)
        outs = sb.tile([64, 7 * B], DT)  # output channels staging

        # --- PSUM tiles ---
        psA = ps.tile([64, 4 * B], DT)  # w_dist @ [f0|f1|f2|f3]
        psB = ps.tile([64, B + 2], DT)  # [w_dist @ masses | rowsum | pad]
        psY = [ps.tile([64, B], DT, name=f"psY{i}") for i in range(1, 5)]

        # --- DMAs in (emitted chunk-major so early chunks land first) ---
        for k in range(4):
            nc.sync.dma_start(fr[:, k * B:(k + 1) * B], fr_d[k, :, :])
        for c in range(KC):
            for k in range(F):
                nc.sync.dma_start(
                    ft[:, k * N + c * 128: k * N + (c + 1) * 128],
                    ft_d[k, c * 128:(c + 1) * 128, :],
                )
            for i in range(5):
                nc.sync.dma_start(
                    wt[:, (i * KC + c) * NS: (i * KC + c + 1) * NS],
                    wt_d[i, c * 128:(c + 1) * 128, :],
                )

        # --- masses (full, transposed layout) = f3^2 - f2^2 - f1^2 - f0^2 ---
        # squares of features 0..3 (ACT does half, DVE the other half)
        nc.scalar.activation(sq[:, 0: 2 * N], ft[:, 0: 2 * N].bitcast(DT), ACTF.Square)
        nc.vector.tensor_tensor(
            out=sq[:, 2 * N: 4 * N],
            in0=ft[:, 2 * N: 4 * N].bitcast(DT),
            in1=ft[:, 2 * N: 4 * N].bitcast(DT),
            op=ALU.mult,
        )
        m3 = mass[:].rearrange("p (c x) -> p c x", c=KC, x=130)
        mview = m3[:, :, 0:128]
        sq4 = sq[:].rearrange("p (k c b) -> p k c b", k=4, c=KC, b=B)
        nc.vector.tensor_tensor(out=mview, in0=sq4[:, 3], in1=sq4[:, 2], op=ALU.subtract)
        nc.vector.tensor_tensor(out=mview, in0=mview, in1=sq4[:, 1], op=ALU.subtract)
        nc.vector.tensor_tensor(out=mview, in0=mview, in1=sq4[:, 0], op=ALU.subtract)
        nc.vector.memset(onez[:, 0:1], 1.0)
        nc.vector.memset(onez[:, 1:2], 0.0)
        nc.vector.tensor_copy(m3[:, :, 128:130], onez[:, None, :].to_broadcast([128, KC, 2]))

        # --- this core's row-slice: masses_R (ch0) and ptsq (ch1) ---
        nc.vector.tensor_tensor(out=frsq[:], in0=fr[:], in1=fr[:], op=ALU.mult)
        nc.vector.tensor_tensor(
            out=outs[:, 0:B], in0=frsq[:, 3 * B: 4 * B], in1=frsq[:, 2 * B: 3 * B],
            op=ALU.subtract,
        )
        nc.vector.tensor_tensor(
            out=outs[:, 0:B], in0=outs[:, 0:B], in1=frsq[:, B: 2 * B], op=ALU.subtract
        )
        nc.vector.tensor_tensor(
            out=outs[:, 0:B], in0=outs[:, 0:B], in1=frsq[:, 0:B], op=ALU.subtract
        )
        nc.vector.tensor_tensor(
            out=outs[:, B: 2 * B], in0=frsq[:, B: 2 * B], in1=frsq[:, 2 * B: 3 * B],
            op=ALU.add,
        )

        # --- matmuls: contraction over m in 4 chunks of 128 ---
        def wslice(i, c):
            return wt[:, (i * KC + c) * NS: (i * KC + c + 1) * NS]

        def fslice(k, c):
            return ft[:, k * N + c * 128: k * N + (c + 1) * 128]

        ft4 = ft[:].rearrange("p (k c b) -> p k c b", k=F, c=KC, b=B)
        for c in range(KC):
            st, sp = c == 0, c == KC - 1
            # w_dist @ [f0|f1|f2|f3] -> psA, one 512-wide moving operand
            nc.tensor.matmul(
                psA[:], wslice(0, c), ft4[:, 0:4, c, :], start=st, stop=sp
            )
            # plain mixes
            for i in range(1, 5):
                nc.tensor.matmul(
                    psY[i - 1][:], wslice(i, c), fslice(MIX_FEAT[i], c),
                    start=st, stop=sp,
                )
        # w_dist @ [masses | ones] (needs masses, so emitted after; own group)
        for c in range(KC):
            nc.tensor.matmul(
                psB[:], wslice(0, c),
                mass[:, c * 130:(c + 1) * 130],
                start=c == 0, stop=c == KC - 1,
            )

        # --- epilogue ---
        # quad_k = fr_k * (w_dist @ f_k), k-major layouts line up
        nc.vector.tensor_tensor(out=quad[:], in0=fr[:], in1=psA[:], op=ALU.mult)
        nc.vector.tensor_tensor(
            out=qs[:, 0:B], in0=quad[:, 0:B], in1=quad[:, B: 2 * B], op=ALU.add
        )
        nc.vector.tensor_tensor(
            out=qs[:, B: 2 * B], in0=quad[:, 2 * B: 3 * B], in1=quad[:, 3 * B: 4 * B],
            op=ALU.subtract,
        )
        nc.vector.tensor_tensor(
            out=qs[:, 0:B], in0=qs[:, 0:B], in1=qs[:, B: 2 * B], op=ALU.add
        )
        # wd = masses_R * rowsum + (w_dist @ masses)
        nc.vector.scalar_tensor_tensor(
            out=qs[:, B: 2 * B],
            in0=outs[:, 0:B],
            scalar=psB[:, B: B + 1],
            in1=psB[:, 0:B],
            op0=ALU.mult,
            op1=ALU.add,
        )
        # ch3 = wd + 2*quad
        nc.vector.scalar_tensor_tensor(
            out=outs[:, 3 * B: 4 * B],
            in0=qs[:, 0:B],
            scalar=2.0,
            in1=qs[:, B: 2 * B],
            op0=ALU.mult,
            op1=ALU.add,
        )
        # plain mix channels: PSUM -> SBUF staging
        for i in range(1, 5):
            ch = MIX_CHAN[i]
            nc.scalar.copy(outs[:, ch * B:(ch + 1) * B], psY[i - 1][:])

        # --- DMAs out, one per channel ---
        for ch in range(7):
            nc.sync.dma_start(out_d[ch, :, :], outs[:, ch * B:(ch + 1) * B])


_NC_CACHE = {}


def _get_nc():
    if "nc" not in _NC_CACHE:
        nc = bacc.Bacc(
            "TRN2", target_bir_lowering=False, debug=False, num_devices=NCORES
        )
        ft_d = nc.dram_tensor("ft", [F, N, B], DTR, kind="ExternalInput")
        wt_d = nc.dram_tensor("wt", [5, N, NS], DTR, kind="ExternalInput")
        fr_d = nc.dram_tensor("fr", [4, NS, B], DT, kind="ExternalInput")
        out_d = nc.dram_tensor("out", [7, NS, B], DT, kind="ExternalOutput")
        with tile.TileContext(nc) as tc:
            _emit(tc, nc, ft_d.ap(), wt_d.ap(), fr_d.ap(), out_d.ap())
        nc.compile()
        _NC_CACHE["nc"] = nc
    return _NC_CACHE["nc"]


def make_in_maps(combvec, w_dist, w_ener, w_pid, w_extra0, w_extra1):
    ftr = np.ascontiguousarray(np.transpose(combvec, (2, 1, 0)), dtype=np.float32)
    weights = (w_dist, w_ener, w_pid, w_extra0, w_extra1)
    in_maps = []
    for c in range(NCORES):
        sl = slice(NS * c, NS * (c + 1))
        wtc = np.stack(
            [np.ascontiguousarray(np.asarray(w, np.float32)[sl].T) for w in weights]
        )
        frc = np.ascontiguousarray(ftr[:4, sl, :])
        in_maps.append({"ft": ftr, "wt": wtc, "fr": frc})
    return in_maps


def kernel(combvec, w_dist, w_ener, w_pid, w_extra0, w_extra1, _bench=None):
    in_maps = make_in_maps(combvec, w_dist, w_ener, w_pid, w_extra0, w_extra1)
    nc = _get_nc()
    kw = dict(_bench) if _bench else {}
    res = run_bass_kernel_spmd(nc, in_maps, core_ids=list(range(NCORES)), **kw)
    full = np.concatenate([r["out"] for r in res.results], axis=1)  # (7, 512, 128)
    out = np.ascontiguousarray(np.transpose(full, (2, 1, 0)))  # (128, 512, 7)
    if _bench is not None:
        kernel.last_results = res
    return out


# revision 10
# speedup vs baseline: 1.6665x; 1.4613x over previous
"""LoLa message-passing kernel for 8 Trainium2 NeuronCores.

Math (algebraically identical to the reference):
  ch0 masses      = f3^2 - f0^2 - f1^2 - f2^2
  ch1 ptsq        = f1^2 + f2^2
  ch2 w_ener@f0, ch4 w_pid@f3, ch5 w_extra0@f4, ch6 w_extra1@f5
  ch3 weighted_d  = masses * rowsum(w_dist) + w_dist @ masses
                    + 2*(f0*(w_dist@f0) + f1*(w_dist@f1)
                         + f2*(w_dist@f2) - f3*(w_dist@f3))

Sharding: model-parallel over particles N (64 output rows per core); combvec
replicated (full contraction operand), weights sliced 1/8 per core.

Device-side design notes:
 - All matmul operands are host-pre-transposed AND host-pre-tiled into
   SBUF-native (128 x cols) layouts so every DMA row is a multi-KB
   contiguous run (HWDGE descriptor generation costs ~5ns/row).
 - fp32 matmul on trn2 is 4-pass; instead operands are split hi/lo into
   bfloat16 and each product computed as Wh@vh + Wh@vl + Wl@vh
   (~3e-6 rel error, 3 single-pass bf16 matmuls, FWL weight loads).
 - The 128x128 PE array computes all 128 output partitions per streamed
   column for free, so two 64-row weight slices are packed side by side
   in one stationary load:
     MM-A: [w_dist | w_ener]  @ [f0|f1|f2|f3]        (512 cols)
     MM-B: [w_pid  | w_extra0]@ [f3|f4]              (256 cols)
     MM-C: [w_dist | w_extra1]@ [f5|masses|1,pad]    (258 cols)
   taking the row/column segments that pair each weight with its feature.
"""

import sys

if "/opt/trn_rl_repo" not in sys.path:
    sys.path.insert(0, "/opt/trn_rl_repo")

import numpy as np
import ml_dtypes

import concourse.bass as bass
import concourse.mybir as mybir
import concourse.tile as tile
from concourse import bacc
from concourse.bass_utils import run_bass_kernel_spmd

B, N, F = 128, 512, 6
NCORES = 8
NS = N // NCORES  # 64 output rows per core
KC = N // 128  # 4 contraction chunks of 128
CW = 1024  # ft tile free-size per chunk: 6*128 feats | 128 masses | 2 ones | pad
DT = mybir.dt.float32
BF = mybir.dt.bfloat16
ALU = mybir.AluOpType
ACTF = mybir.ActivationFunctionType

# stationary pairs, each stored contiguously (128 cols per pair, w_dist twice):
# pair 0: [w_dist|w_ener], pair 1: [w_pid|w_extra0], pair 2: [w_dist|w_extra1]
W_PAIRS = (("w_dist", "w_ener"), ("w_pid", "w_extra0"), ("w_dist", "w_extra1"))
PW = 3 * 128  # wt tile free-size per chunk


def _emit(tc, nc, fth_d, ftl_d, wth_d, wtl_d, fr_d, out_d):
    with (
        tc.tile_pool(name="sbuf", bufs=1) as sb,
        tc.tile_pool(name="scratch", bufs=2) as scr,
        tc.tile_pool(name="psum", bufs=1, space="PSUM") as ps,
    ):
        # --- persistent SBUF tiles ---
        fth = sb.tile([128, KC * CW], BF)  # hi: [c*1024 + k*128 | m | ones]
        ftl = sb.tile([128, KC * CW], BF)  # lo
        wth = sb.tile([128, KC * PW], BF)  # hi weight pairs [c*384 + j*128 + n]
        wtl = sb.tile([128, KC * PW], BF)  # lo
        fr = sb.tile([64, 4 * B], DT)  # this core's n-rows of f0..f3 (fp32)
        frsq = sb.tile([64, 4 * B], DT)
        quad = sb.tile([64, 4 * B], DT)
        qs = sb.tile([64, 2 * B], DT)
        olo = sb.tile([64, 4 * B], DT)  # out staging: ch 0,1,3,4 (partitions 0:64)
        ohi = sb.tile([128, 3 * B], DT)  # out staging: ch 2,5,6 (partitions 64:128)

        # --- DMAs in: ft chunks on sync, weights + fr on scalar ---
        for c in range(KC):
            nc.sync.dma_start(
                fth[:, c * CW: c * CW + 768], fth_d[:, c * 768:(c + 1) * 768]
            )
            nc.sync.dma_start(
                ftl[:, c * CW: c * CW + 768], ftl_d[:, c * 768:(c + 1) * 768]
            )
        nc.scalar.dma_start(wth[:], wth_d[:])
        nc.scalar.dma_start(wtl[:], wtl_d[:])
        nc.scalar.dma_start(fr[:], fr_d[:])

        # ones columns: hi=1.0, lo=0.0 at [c*CW + 896 : +898]
        fth4 = fth[:].rearrange("p (c x) -> p c x", c=KC, x=CW)
        ftl4 = ftl[:].rearrange("p (c x) -> p c x", c=KC, x=CW)
        nc.vector.memset(fth4[:, :, 896:898], 1.0)
        nc.vector.memset(ftl4[:, :, 896:898], 0.0)

        # --- per-chunk masses: recon f32, square, combine, split hi/lo ---
        for c in range(KC):
            base = c * CW
            rec = scr.tile([128, 4 * B], DT, name="rec")
            sq = scr.tile([128, 4 * B], DT, name="sq")
            mss = scr.tile([128, B], DT, name="mss")
            # f = fh + fl for features 0..3
            nc.vector.tensor_tensor(
                out=rec[:], in0=fth[:, base: base + 512],
                in1=ftl[:, base: base + 512], op=ALU.add,
            )
            nc.scalar.activation(sq[:], rec[:], ACTF.Square)
            nc.vector.tensor_tensor(
                out=mss[:], in0=sq[:, 3 * B: 4 * B], in1=sq[:, 2 * B: 3 * B],
                op=ALU.subtract,
            )
            nc.vector.tensor_tensor(
                out=mss[:], in0=mss[:], in1=sq[:, B: 2 * B], op=ALU.subtract
            )
            nc.vector.tensor_tensor(
                out=mss[:], in0=mss[:], in1=sq[:, 0:B], op=ALU.subtract
            )
            # split: mh = bf16(m); ml = bf16(m - mh)
            nc.vector.tensor_copy(fth[:, base + 768: base + 896], mss[:])
            nc.vector.tensor_tensor(
                out=ftl[:, base + 768: base + 896], in0=mss[:],
                in1=fth[:, base + 768: base + 896], op=ALU.subtract,
            )

        # --- this core's row-slice: masses_R (ch0) and ptsq (ch1), fp32 ---
        nc.vector.tensor_tensor(out=frsq[:], in0=fr[:], in1=fr[:], op=ALU.mult)
        nc.vector.tensor_tensor(
            out=olo[:, 0:B], in0=frsq[:, 3 * B: 4 * B], in1=frsq[:, 2 * B: 3 * B],
            op=ALU.subtract,
        )
        nc.vector.tensor_tensor(
            out=olo[:, 0:B], in0=olo[:, 0:B], in1=frsq[:, B: 2 * B], op=ALU.subtract
        )
        nc.vector.tensor_tensor(
            out=olo[:, 0:B], in0=olo[:, 0:B], in1=frsq[:, 0:B], op=ALU.subtract
        )
        nc.vector.tensor_tensor(
            out=olo[:, B: 2 * B], in0=frsq[:, B: 2 * B], in1=frsq[:, 2 * B: 3 * B],
            op=ALU.add,
        )

        # --- PSUM tiles ---
        psA = ps.tile([128, 512], DT)  # [dist|ener] @ [f0|f1|f2|f3]
        psB = ps.tile([128, 256], DT)  # [pid|x0]   @ [f3|f4]
        psC = ps.tile([128, 258], DT)  # [dist|x1]  @ [f5|m|1,pad]

        # --- matmuls: 3 logical per chunk x 3 bf16 terms, accumulated ---
        def stats(c, j):
            off = c * PW + j * 128
            return wth[:, off: off + 128], wtl[:, off: off + 128]

        for c in range(KC):
            base = c * CW
            plan = (
                (psA, stats(c, 0), base, 512),
                (psB, stats(c, 1), base + 384, 256),
                (psC, stats(c, 2), base + 640, 258),
            )
            for pst, (sh, sl_), off, ln in plan:
                for t, (stat, mov) in enumerate(
                    ((sh, fth), (sh, ftl), (sl_, fth))
                ):
                    nc.tensor.matmul(
                        pst[:], stat, mov[:, off: off + ln],
                        start=(c == 0 and t == 0),
                        stop=(c == KC - 1 and t == 2),
                    )

        # --- epilogue ---
        # psA[0:64]        = w_dist@[f0|f1|f2|f3] (cols k*128+b)
        # psA[64:128,0:128]= w_ener@f0 -> ch2 (hi)
        # psB[0:64, 0:128] = w_pid@f3  -> ch4 (lo)
        # psB[64:128,128:] = w_x0@f4   -> ch5 (hi)
        # psC[0:64,128:256]= w_dist@m;  psC[0:64,256] = rowsum (lo)
        # psC[64:128,0:128]= w_x1@f5   -> ch6 (hi)
        nc.vector.tensor_tensor(out=quad[:], in0=fr[:], in1=psA[0:64, :], op=ALU.mult)
        nc.vector.tensor_tensor(
            out=qs[:, 0:B], in0=quad[:, 0:B], in1=quad[:, B: 2 * B], op=ALU.add
        )
        nc.vector.tensor_tensor(
            out=qs[:, B: 2 * B], in0=quad[:, 2 * B: 3 * B], in1=quad[:, 3 * B: 4 * B],
            op=ALU.subtract,
        )
        nc.vector.tensor_tensor(
            out=qs[:, 0:B], in0=qs[:, 0:B], in1=qs[:, B: 2 * B], op=ALU.add
        )
        nc.vector.scalar_tensor_tensor(
            out=qs[:, B: 2 * B],
            in0=olo[:, 0:B],
            scalar=psC[0:64, 256:257],
            in1=psC[0:64, 128:256],
            op0=ALU.mult,
            op1=ALU.add,
        )
        nc.vector.scalar_tensor_tensor(
            out=olo[:, 2 * B: 3 * B],
            in0=qs[:, 0:B],
            scalar=2.0,
            in1=qs[:, B: 2 * B],
            op0=ALU.mult,
            op1=ALU.add,
        )
        # ch4 = w_pid@f3 (low partitions)
        nc.scalar.copy(olo[:, 3 * B: 4 * B], psB[0:64, 0:B])
        # high-partition channels
        nc.scalar.copy(ohi[64:128, 0:B], psA[64:128, 0:B])  # ch2 ener
        nc.scalar.copy(ohi[64:128, B: 2 * B], psB[64:128, B: 2 * B])  # ch5 x0
        nc.scalar.copy(ohi[64:128, 2 * B: 3 * B], psC[64:128, 0:B])  # ch6 x1

        # --- DMAs out: olo cols [0:512] = ch 0,1,3,4; ohi cols [512:896] ---
        nc.sync.dma_start(out_d[:, 0: 4 * B], olo[:])
        nc.sync.dma_start(out_d[:, 4 * B: 7 * B], ohi[64:128, :])


_NC_CACHE = {}


def _get_nc():
    if "nc" not in _NC_CACHE:
        nc = bacc.Bacc(
            "TRN2", target_bir_lowering=False, debug=False, num_devices=NCORES
        )
        fth_d = nc.dram_tensor("fth", [128, KC * 768], BF, kind="ExternalInput")
        ftl_d = nc.dram_tensor("ftl", [128, KC * 768], BF, kind="ExternalInput")
        wth_d = nc.dram_tensor("wth", [128, KC * PW], BF, kind="ExternalInput")
        wtl_d = nc.dram_tensor("wtl", [128, KC * PW], BF, kind="ExternalInput")
        fr_d = nc.dram_tensor("fr", [64, 4 * B], DT, kind="ExternalInput")
        out_d = nc.dram_tensor("out", [64, 7 * B], DT, kind="ExternalOutput")
        with tile.TileContext(nc) as tc:
            _emit(
                tc, nc, fth_d.ap(), ftl_d.ap(), wth_d.ap(), wtl_d.ap(),
                fr_d.ap(), out_d.ap(),
            )
        nc.compile()
        _NC_CACHE["nc"] = nc
    return _NC_CACHE["nc"]


def make_in_maps(combvec, w_dist, w_ener, w_pid, w_extra0, w_extra1):
    ft_t = np.ascontiguousarray(
        np.transpose(np.asarray(combvec, np.float32), (2, 1, 0))
    )  # (6, 512, 128) [k, m, b]
    # ft layout: [p, c*768 + k*128 + b] = ft_t[k, c*128+p, b]
    ftfull = np.ascontiguousarray(
        ft_t.reshape(F, KC, 128, B).transpose(2, 1, 0, 3)
    ).reshape(128, KC * 768)
    fth_np = ftfull.astype(ml_dtypes.bfloat16)
    ftl_np = (ftfull - fth_np.astype(np.float32)).astype(ml_dtypes.bfloat16)

    weights = {
        "w_dist": np.asarray(w_dist, np.float32),
        "w_pid": np.asarray(w_pid, np.float32),
        "w_ener": np.asarray(w_ener, np.float32),
        "w_extra0": np.asarray(w_extra0, np.float32),
        "w_extra1": np.asarray(w_extra1, np.float32),
    }
    in_maps = []
    for core in range(NCORES):
        sl = slice(NS * core, NS * (core + 1))
        # wt layout: [p, c*384 + j*128 + s*64 + n] = pair_j[s][64*core+n, c*128+p]
        stk = np.stack(
            [
                np.stack(
                    [weights[a][sl].T.reshape(KC, 128, NS),
                     weights[b][sl].T.reshape(KC, 128, NS)], axis=2
                )  # (c, p, s, n)
                for a, b in W_PAIRS
            ]
        )  # (j, c, p, s, n)
        wt = np.ascontiguousarray(stk.transpose(2, 1, 0, 3, 4)).reshape(128, KC * PW)
        wth_np = wt.astype(ml_dtypes.bfloat16)
        wtl_np = (wt - wth_np.astype(np.float32)).astype(ml_dtypes.bfloat16)
        # fr layout: [p, k*128 + b] = ft_t[k, 64*core+p, b], fp32
        frc = np.ascontiguousarray(
            ft_t[:4, sl, :].transpose(1, 0, 2)
        ).reshape(NS, 4 * B)
        in_maps.append(
            {"fth": fth_np, "ftl": ftl_np, "wth": wth_np, "wtl": wtl_np, "fr": frc}
        )
    return in_maps


# out channel order in the DRAM out tensor columns
OUT_ORDER = [0, 1, 3, 4, 2, 5, 6]


def assemble(results):
    full = np.empty((B, N, 7), np.float32)
    for core, r in enumerate(results):
        o = r["out"].reshape(NS, 7, B)  # (n, slot, b)
        for slot, ch in enumerate(OUT_ORDER):
            full[:, NS * core: NS * (core + 1), ch] = o[:, slot, :].T
    return full


def kernel(combvec, w_dist, w_ener, w_pid, w_extra0, w_extra1, _bench=None):
    in_maps = make_in_maps(combvec, w_dist, w_ener, w_pid, w_extra0, w_extra1)
    nc = _get_nc()
    kw = dict(_bench) if _bench else {}
    res = run_bass_kernel_spmd(nc, in_maps, core_ids=list(range(NCORES)), **kw)
    out = assemble(res.results)
    if _bench is not None:
        kernel.last_results = res
    return out
